# revision 1
# baseline (speedup 1.0000x reference)
# Conv2d 3x3 VALID stride-1 as implicit GEMM on 8 TRN2 NeuronCores.
#
# Problem: x[32,128,56,56] f32, weight[256,128,3,3] f32, bias[256] f32
#          -> out[32,256,54,54] f32
#
# Sharding: data-parallel over batch — 4 images per core, weight replicated.
# Per-core kernel: for each (image, oc-half, 9-output-row unit) accumulate
# the 9 kernel-position matmuls into one PSUM bank (contraction dim = 128
# input channels on the partition axis; N = 9*54 = 486 <= 512 fits a bank),
# evict through ScalarE (bias add) and DMA out.
# Compute dtype: fp16 (PE runs fp16 at 1 cycle/row vs 4 for fp32; PSUM
# accumulation is fp32; measured rel err vs f32 reference ~3e-4).
#
# Startup: DMAs are issued so the first unit's dependencies (x image-0
# rows 0..10 + oc-half-0 weights) land first; dummy matmuls on a scratch
# tile keep the PE busy from t~0 so the HAM clock-gate lifts (1.2->2.4 GHz)
# before the real matmuls begin.

import numpy as np

import concourse.tile as tile
from concourse import bacc, mybir
from concourse.bass_utils import run_bass_kernel_spmd

N_CORES = 8
IMGS = 4          # images per core
IC = 128
OC = 256
H = W = 56
OH = OW = 54
KH = KW = 3
CHUNK_ROWS = 9    # output rows per unit (N = 9*54 = 486 <= 512, one bank)
NTILE = OH // CHUNK_ROWS
NPOS = CHUNK_ROWS * OW

FP16 = mybir.dt.float16
F32 = mybir.dt.float32

N_WARMUP_MM = 60  # dummy matmuls to lift the PE HAM clock gate


def build_conv_bass(repeat=1, num_devices=N_CORES, variant=0):
    # variant 0: production — unit-serial, one LDWEIGHTS per matmul
    # variant 1: weight-stationary groups of 3 units (LDW per 3 MMs,
    #            PSUM-bank-interleaved matmul stream)
    # variant 2: diagnostic MM-only ablation (evict/DMA only for the last
    #            unit; output is WRONG — timing use only)
    # variant 3: batched out-DMA — evict 6 units into one staging block,
    #            single big DMA per (img, och)
    # variant 4: variant 3 + alternate ACT/DVE evictions
    # variant 5: variant 0 + alternate ACT/DVE evictions
    nc = bacc.Bacc(
        "TRN2",
        target_bir_lowering=False,
        debug=False,
        num_devices=num_devices,
    )
    x_ext = nc.dram_tensor("x", [IMGS, IC, H, W], FP16, kind="ExternalInput")
    wt_ext = nc.dram_tensor("wt", [2, IC, KH, KW, 128], FP16, kind="ExternalInput")
    b_ext = nc.dram_tensor("bias", [128, 2], F32, kind="ExternalInput")
    out_ext = nc.dram_tensor("out", [IMGS, OC, OH, OW], F32, kind="ExternalOutput")

    with tile.TileContext(nc) as tc:
        with (
            tc.tile_pool(name="consts", bufs=1) as cpool,
            tc.tile_pool(name="xin", bufs=IMGS) as xpool,
            tc.tile_pool(name="psum", bufs=6, space="PSUM") as ppool,
            tc.tile_pool(name="warm", bufs=1, space="PSUM") as wpsum,
            tc.tile_pool(name="outs", bufs=4) as opool,
            tc.tile_pool(name="oblk", bufs=2) as oblkpool,
        ):
            # PE warm-up: matmuls on a zeroed scratch tile, no DMA deps.
            warm_in = cpool.tile([128, 128], FP16)
            nc.vector.memset(warm_in[:], 0.0)
            warm_ps = wpsum.tile([128, 64], F32)
            for _ in range(N_WARMUP_MM):
                nc.tensor.matmul(warm_ps[:], warm_in[:], warm_in[:, 0:64],
                                 start=True, stop=True)

            # Startup-ordered DMAs: first unit's deps first (x0 rows 0..10 +
            # oc-half-0 weights), then the rest in need order.
            x_tiles = [xpool.tile([IC, H, W], FP16, tag=f"x{i}", name=f"x{i}")
                       for i in range(IMGS)]
            w_sbs = [cpool.tile([IC, KH, KW, 128], FP16, tag=f"w{och}",
                                name=f"w{och}")
                     for och in range(2)]
            nc.sync.dma_start(w_sbs[0][:], wt_ext[0])
            nc.sync.dma_start(x_tiles[0][:, 0:11], x_ext[0, :, 0:11])
            nc.sync.dma_start(x_tiles[0][:, 11:20], x_ext[0, :, 11:20])
            nc.sync.dma_start(x_tiles[0][:, 20:H], x_ext[0, :, 20:H])
            nc.sync.dma_start(w_sbs[1][:], wt_ext[1])
            b_sb = cpool.tile([128, 2], F32)
            nc.sync.dma_start(b_sb[:], b_ext[:])
            for img in range(1, IMGS):
                nc.sync.dma_start(x_tiles[img][:], x_ext[img])

            def evict_and_store(ps, img, och, t):
                ob = opool.tile([128, NPOS], F32, tag="ob",
                                name=f"ob{img}_{och}_{t}")
                nc.scalar.activation(
                    ob[:],
                    ps[:, 0:NPOS],
                    mybir.ActivationFunctionType.Identity,
                    bias=b_sb[:, och:och + 1],
                )
                nc.sync.dma_start(
                    out_ext[
                        img,
                        och * 128:(och + 1) * 128,
                        t * CHUNK_ROWS:(t + 1) * CHUNK_ROWS,
                        :,
                    ],
                    ob[:],
                )

            def evict_into(dst_ap, ps, och, use_dve):
                if use_dve:
                    nc.vector.tensor_scalar_add(
                        dst_ap, ps[:, 0:NPOS], b_sb[:, och:och + 1])
                else:
                    nc.scalar.activation(
                        dst_ap, ps[:, 0:NPOS],
                        mybir.ActivationFunctionType.Identity,
                        bias=b_sb[:, och:och + 1])

            for _rep in range(repeat):
              for img in range(IMGS):
                for och in range(2):
                  # Batched out-DMA (default): evict the 6 units of one
                  # (img, och) block into a staging tile, one big DMA per
                  # block. The final block keeps per-unit DMAs so the
                  # kernel tail is one small transfer, not a 1.5 MB one.
                  is_final_blk = (_rep == repeat - 1 and img == IMGS - 1
                                  and och == 1)
                  if variant in (0, 3, 4) and not is_final_blk:
                    ob_blk = oblkpool.tile([128, NTILE, NPOS], F32, tag="obb",
                                           name=f"obb{img}_{och}")
                    for t in range(NTILE):
                        ps = ppool.tile([128, 512], F32, tag="ps",
                                        name=f"psb{t}")
                        r0 = t * CHUNK_ROWS
                        for kh in range(KH):
                            for kw in range(KW):
                                nc.tensor.matmul(
                                    ps[:, 0:NPOS],
                                    w_sbs[och][:, kh, kw, :],
                                    x_tiles[img][
                                        :, r0 + kh:r0 + kh + CHUNK_ROWS,
                                        kw:kw + OW
                                    ],
                                    start=(kh == 0 and kw == 0),
                                    stop=(kh == KH - 1 and kw == KW - 1),
                                )
                        evict_into(ob_blk[:, t], ps, och,
                                   use_dve=(variant == 4 and t % 2 == 1))
                    nc.sync.dma_start(
                        out_ext[img, och * 128:(och + 1) * 128, :, :],
                        ob_blk[:],
                    )
                  elif variant == 1:
                    for g in range(NTILE // 3):
                        pss = [ppool.tile([128, 512], F32, tag="ps",
                                          name=f"ps{g}_{u}")
                               for u in range(3)]
                        for kh in range(KH):
                            for kw in range(KW):
                                for u in range(3):
                                    t = g * 3 + u
                                    r0 = t * CHUNK_ROWS
                                    nc.tensor.matmul(
                                        pss[u][:, 0:NPOS],
                                        w_sbs[och][:, kh, kw, :],
                                        x_tiles[img][
                                            :, r0 + kh:r0 + kh + CHUNK_ROWS,
                                            kw:kw + OW
                                        ],
                                        start=(kh == 0 and kw == 0),
                                        stop=(kh == KH - 1 and kw == KW - 1),
                                    )
                        for u in range(3):
                            evict_and_store(pss[u], img, och, g * 3 + u)
                  else:
                    for t in range(NTILE):
                        ps = ppool.tile([128, 512], F32)
                        r0 = t * CHUNK_ROWS
                        for kh in range(KH):
                            for kw in range(KW):
                                nc.tensor.matmul(
                                    ps[:, 0:NPOS],
                                    w_sbs[och][:, kh, kw, :],
                                    x_tiles[img][
                                        :, r0 + kh:r0 + kh + CHUNK_ROWS,
                                        kw:kw + OW
                                    ],
                                    start=(kh == 0 and kw == 0),
                                    stop=(kh == KH - 1 and kw == KW - 1),
                                )
                        is_final = (img == IMGS - 1 and och == 1
                                    and t == NTILE - 1)
                        if variant == 5:
                            ob = opool.tile([128, NPOS], F32, tag="ob",
                                            name=f"ob5_{img}_{och}_{t}")
                            evict_into(ob[:], ps, och, use_dve=(t % 2 == 1))
                            nc.sync.dma_start(
                                out_ext[
                                    img,
                                    och * 128:(och + 1) * 128,
                                    r0:r0 + CHUNK_ROWS,
                                    :,
                                ],
                                ob[:],
                            )
                        elif variant != 2 or is_final:
                            evict_and_store(ps, img, och, t)
    nc.compile()
    return nc


_CACHE = {}


def _get_nc(repeat=1, variant=0):
    key = (repeat, variant)
    if key not in _CACHE:
        _CACHE[key] = build_conv_bass(repeat=repeat, variant=variant)
    return _CACHE[key]


def kernel(x, weight, bias, _want_results_obj=False, _repeat=1, **run_kwargs):
    x = np.asarray(x)
    weight = np.asarray(weight)
    bias = np.asarray(bias)
    assert x.shape == (32, IC, H, W)
    x16 = np.ascontiguousarray(x.astype(np.float16))
    # weight [oc, ic, kh, kw] -> [och, ic, kh, kw, oc_lo] for the two
    # oc-half weight tiles
    wt = np.ascontiguousarray(
        weight.astype(np.float16)
        .transpose(1, 2, 3, 0)            # [ic, kh, kw, oc]
        .reshape(IC, KH, KW, 2, 128)
        .transpose(3, 0, 1, 2, 4)         # [och, ic, kh, kw, 128]
    )
    b2 = np.ascontiguousarray(
        bias.astype(np.float32).reshape(2, 128).T
    )  # [128, 2]: b2[p, h] = bias[h*128+p]

    nc = _get_nc(_repeat)
    in_maps = [
        {"x": x16[i * IMGS:(i + 1) * IMGS], "wt": wt, "bias": b2}
        for i in range(N_CORES)
    ]
    res = run_bass_kernel_spmd(nc, in_maps, core_ids=list(range(N_CORES)), **run_kwargs)
    out = np.concatenate([res.results[i]["out"] for i in range(N_CORES)], axis=0)
    if _want_results_obj:
        return out, res
    return out

